# revision 1
# baseline (speedup 1.0000x reference)
"""Trainium2 Bass kernel for EnhancedGNNModel (3-layer GNN message passing).

Strategy (8 NeuronCores, SPMD):
  - Nodes are sharded by dst range: core c owns rows [c*NPC, (c+1)*NPC).
  - Edges are sharded by dst: each core aggregates messages for its own nodes.
  - Per-edge message Linear(concat(h_src, h_dst)) @ W is split algebraically:
        msgs = h[src] @ W_top + h[dst] @ W_bot + b
    so segment_sum(msgs, dst) = (sum_{e->d} h[src]) @ W_top
                                + deg[d] * (h[d] @ W_bot) + deg[d] * b.
    Only S[d] = sum of gathered h[src] rows needs edge-granularity work.
  - S is built per 128-node dst block: dma_gather pulls h[src] rows into a
    [128 edges, HID] SBUF tile, a one-hot(dst_rel) matrix built on the vector
    engine turns the scatter-add into a PE matmul accumulated in PSUM.
  - h lives transposed ([HID, node]) in SBUF for all dense matmuls; updated
    rows are transposed back via the PE, written to DRAM, and AllGathered so
    every core has the full h for the next layer's gathers.
"""
import hashlib
import ml_dtypes
import numpy as np
from contextlib import ExitStack
from dataclasses import dataclass

import concourse.bass as bass
import concourse.tile as tile
from concourse import bacc, mybir
from concourse.bass_utils import run_bass_kernel_spmd

F32 = mybir.dt.float32
BF16 = mybir.dt.bfloat16
I16 = mybir.dt.int16


@dataclass(frozen=True)
class Cfg:
    n_nodes: int = 50000
    n_edges: int = 800000
    feat: int = 64
    hid: int = 128
    layers: int = 3
    n_cores: int = 8
    gcap: int = 1024  # max indices per dma_gather call
    bf16_h: bool = True  # store/gather h in bf16 (message path only)
    dma_scratch: int = 16384  # SWDGE descriptor-ring carveout bytes/partition
    n_queues: int = 1  # SWDGE queues for dma_gather

    @property
    def npc(self):  # nodes per core (multiple of 128)
        per = -(-self.n_nodes // self.n_cores)
        return -(-per // 128) * 128

    @property
    def n_pad(self):
        return self.npc * self.n_cores

    @property
    def bank_rows(self):
        assert self.n_pad % 2 == 0
        b = self.n_pad // 2
        assert b <= 32768, "dma_gather int16 index limit"
        return b

    @property
    def n_blocks(self):
        return self.npc // 128


def _round_up(x, m):
    return -(-x // m) * m


def plan_edges(cfg: Cfg, edge_indices: np.ndarray):
    """Host-side sharding plan. Returns (meta, per_core) where meta describes
    the (core-invariant) program structure and per_core holds the data arrays.
    """
    src = np.asarray(edge_indices[0], dtype=np.int64)
    dst = np.asarray(edge_indices[1], dtype=np.int64)
    C, npc, nb, br = cfg.n_cores, cfg.npc, cfg.n_blocks, cfg.bank_rows

    core = dst // npc
    # bucket key per edge: (core, block, bank)
    ldst = dst - core * npc
    block = ldst >> 7
    dst_rel = ldst & 127
    bank = (src >= br).astype(np.int64)
    bidx = src - bank * br

    # counts[c, b, k]
    counts = np.zeros((C, nb, 2), dtype=np.int64)
    np.add.at(counts, (core, block, bank), 1)
    caps = counts.max(axis=0)  # [nb, 2]
    caps = np.where(caps > 0, ((caps + 127) // 128) * 128, 0).astype(np.int64)

    # order edges by (core, block, bank) via argsort for grouping
    key = (core * nb + block) * 2 + bank
    order = np.argsort(key, kind="stable")
    src_s, bidx_s, rel_s, key_s = src[order], bidx[order], dst_rel[order], key[order]
    # start offset of each (c,b,k) group in the sorted array
    group_sizes = np.bincount(key_s, minlength=C * nb * 2)
    group_starts = np.concatenate([[0], np.cumsum(group_sizes)])

    total_cap = int(caps.sum())
    n_chunks = total_cap // 128
    assert total_cap % 16 == 0

    per_core = []
    for c in range(C):
        idx_flat = np.zeros(total_cap, dtype=np.int16)
        rel_flat = np.full(total_cap, -1.0, dtype=np.float32)
        off = 0
        for b in range(nb):
            for k in range(2):
                cap = int(caps[b, k])
                if cap == 0:
                    continue
                g = (c * nb + b) * 2 + k
                s, n = group_starts[g], group_sizes[g]
                idx_flat[off:off + n] = bidx_s[s:s + n].astype(np.int16)
                rel_flat[off:off + n] = rel_s[s:s + n].astype(np.float32)
                off += cap
        assert off == total_cap
        per_core.append((idx_flat, rel_flat))

    # wrap indices per gather call (call boundaries at multiples of gcap inside
    # each bucket): idx j of a call sits at [j % 16, j // 16] of its slice.
    calls = []  # (block, first_of_block, bank, cap_call, off16, chunk_base)
    off = 0
    chunk = 0
    for b in range(nb):
        first = True
        for k in range(2):
            cap = int(caps[b, k])
            q = 0
            while q < cap:
                cc = min(cfg.gcap, cap - q)
                calls.append((b, first, k, cc, off // 16, chunk))
                first = False
                off += cc
                chunk += cc // 128
                q += cc
    assert off == total_cap and chunk == n_chunks

    def wrap(flat):
        out = np.zeros((16, total_cap // 16), dtype=flat.dtype)
        for (_, _, _, cc, off16, _) in calls:
            seg = flat[off16 * 16: off16 * 16 + cc].reshape(cc // 16, 16).T
            out[:, off16: off16 + cc // 16] = seg
        return np.tile(out, (8, 1))

    per_core_arr = []
    for c in range(C):
        idx_flat, rel_flat = per_core[c]
        idx_w = wrap(idx_flat)  # [128, total_cap//16] int16
        rel_t = rel_flat.reshape(n_chunks, 128).T.copy()  # [128, n_chunks]
        # degree / mask per local node
        deg = np.zeros(npc, dtype=np.float32)
        m = core == c
        np.add.at(deg, ldst[m], 1.0)
        mask = np.zeros(npc, dtype=np.float32)
        lo = c * npc
        mask[: max(0, min(npc, cfg.n_nodes - lo))] = 1.0
        per_core_arr.append(dict(idxs=idx_w, dstrel=rel_t, mask=mask[None, :],
                                 deg_bcast=np.tile(deg[None, :], (128, 1))))

    meta = dict(caps=caps, calls=calls, n_chunks=n_chunks, total_cap=total_cap)
    return meta, per_core_arr


def build_program(cfg: Cfg, meta):
    C, npc, nb = cfg.n_cores, cfg.npc, cfg.n_blocks
    FEAT, HID, L = cfg.feat, cfg.hid, cfg.layers
    n_chunks, total_cap = meta["n_chunks"], meta["total_cap"]
    calls = meta["calls"]

    HDT = BF16 if cfg.bf16_h else F32
    nc = bacc.Bacc("TRN2", target_bir_lowering=False, debug=False, num_devices=C,
                   dynamic_dma_scratch_size=cfg.dma_scratch,
                   num_swdge_queues=cfg.n_queues)

    def inp(name, shape, dt=F32):
        return nc.dram_tensor(name, shape, dt, kind="ExternalInput").ap()

    at_d = inp("at_own", [FEAT, npc])
    embW_d = inp("emb_W", [FEAT, HID])
    embb_d = inp("emb_b", [1, HID])
    mtop_d = inp("msg_top", [L * HID, HID])
    mbot_d = inp("msg_bot", [L * HID, HID])
    mb_d = inp("msg_bias", [L, HID])
    utop_d = inp("upd_top", [L * HID, HID])
    ubot_d = inp("upd_bot", [L * HID, HID])
    ub_d = inp("upd_bias", [L, HID])
    ow1_d = inp("out_W1", [HID, HID // 2])
    ob1_d = inp("out_b1", [HID // 2, 1])
    ow2_d = inp("out_W2", [HID // 2, HID // 4])
    ob2_d = inp("out_b2", [HID // 4, 1])
    ow3_d = inp("out_W3", [HID // 4, 1])
    ob3_d = inp("out_b3", [1, 1])
    mask_d = inp("mask", [1, npc])
    degb_d = inp("deg_bcast", [128, npc])
    idx_d = inp("idxs", [128, total_cap // 16], I16)
    rel_d = inp("dstrel", [128, n_chunks])
    out_d = nc.dram_tensor("result", [1, 1], F32, kind="ExternalOutput").ap()

    with tile.TileContext(nc) as tc, ExitStack() as ctx:
        const = ctx.enter_context(tc.tile_pool(name="const", bufs=1))
        gpool = ctx.enter_context(tc.tile_pool(name="g", bufs=3))
        ohpool = ctx.enter_context(tc.tile_pool(name="oh", bufs=6))
        spool = ctx.enter_context(tc.tile_pool(name="s", bufs=4))
        rpool = ctx.enter_context(tc.tile_pool(name="rows", bufs=4))
        ps_s = ctx.enter_context(tc.tile_pool(name="ps_s", bufs=2, space="PSUM"))
        ps_d = ctx.enter_context(tc.tile_pool(name="ps_d", bufs=4, space="PSUM"))
        dram = ctx.enter_context(tc.tile_pool(name="dram", bufs=2, space="DRAM"))

        def load_const(name, ap_dram, shape, dt=F32):
            t = const.tile(shape, dt, name=name, tag=name)
            nc.sync.dma_start(out=t[:], in_=ap_dram)
            return t

        embW_sb = load_const("embW_sb", embW_d[:], [FEAT, HID])
        embb_sb = load_const("embb_sb", embb_d[:], [1, HID])
        LW = min(L, 3)  # distinct weight sets resident (L>3 only in timing probes)
        mtop_sb = [load_const(f"mtop{l}", mtop_d[l * HID:(l + 1) * HID, :], [HID, HID]) for l in range(LW)]
        mbot_sb = [load_const(f"mbot{l}", mbot_d[l * HID:(l + 1) * HID, :], [HID, HID]) for l in range(LW)]
        mb_sb = [load_const(f"mb{l}", mb_d[l:l + 1, :], [1, HID]) for l in range(LW)]
        utop_sb = [load_const(f"utop{l}", utop_d[l * HID:(l + 1) * HID, :], [HID, HID]) for l in range(LW)]
        ubot_sb = [load_const(f"ubot{l}", ubot_d[l * HID:(l + 1) * HID, :], [HID, HID]) for l in range(LW)]
        ub_sb = [load_const(f"ub{l}", ub_d[l:l + 1, :], [1, HID]) for l in range(LW)]
        ow1_sb = load_const("ow1_sb", ow1_d[:], [HID, HID // 2])
        ob1_sb = load_const("ob1_sb", ob1_d[:], [HID // 2, 1])
        ow2_sb = load_const("ow2_sb", ow2_d[:], [HID // 2, HID // 4])
        ob2_sb = load_const("ob2_sb", ob2_d[:], [HID // 4, 1])
        ow3_sb = load_const("ow3_sb", ow3_d[:], [HID // 4, 1])
        ob3_sb = load_const("ob3_sb", ob3_d[:], [1, 1])
        mask_sb = load_const("mask_sb", mask_d[:], [1, npc])
        degb_sb = load_const("degb_sb", degb_d[:], [128, npc])
        deg_sb = degb_sb[0:1, :]
        idx_sb = load_const("idx_sb", idx_d[:], [128, total_cap // 16], I16)
        rel_sb = load_const("rel_sb", rel_d[:], [128, n_chunks])

        iota_sb = const.tile([128, 128], F32)
        nc.gpsimd.iota(iota_sb[:], [[1, 128]], channel_multiplier=0,
                       allow_small_or_imprecise_dtypes=True)
        iotah_sb = const.tile([128, 128], HDT)
        nc.gpsimd.iota(iotah_sb[:], [[1, 128]], channel_multiplier=0,
                       allow_small_or_imprecise_dtypes=True)
        iota_col = const.tile([128, 1], F32)
        nc.gpsimd.iota(iota_col[:], [[1, 1]], channel_multiplier=1,
                       allow_small_or_imprecise_dtypes=True)
        ident_sb = const.tile([128, 128], F32)
        nc.vector.tensor_scalar(ident_sb[:], iota_sb[:], iota_col[:], None,
                                op0=mybir.AluOpType.is_equal)

        hT = const.tile([128, npc], F32)  # h transposed, own nodes

        def emit_h_rows():
            """Transpose hT to row-major, DMA to DRAM, AllGather full h."""
            rows_dram = dram.tile([npc, HID], HDT, tag="rows_d")
            for b in range(nb):
                blk = slice(b * 128, (b + 1) * 128)
                pt = ps_d.tile([128, 512], F32, tag="d")
                nc.tensor.transpose(pt[:, :128], hT[:, blk], ident_sb[:])
                r_sb = rpool.tile([128, 128], HDT, tag="r")
                nc.scalar.copy(r_sb[:], pt[:, :128])
                nc.sync.dma_start(out=rows_dram[blk, :], in_=r_sb[:])
            hfull = dram.tile([cfg.n_pad, HID], HDT, tag="hfull_d", addr_space="Shared")
            nc.gpsimd.collective_compute(
                "AllGather", mybir.AluOpType.bypass,
                replica_groups=[list(range(C))],
                ins=[rows_dram.opt()], outs=[hfull.opt()],
            )
            return hfull

        # ---- embedding: hT = emb_W^T @ at_own + emb_b (mask-broadcast) ----
        with tc.tile_pool(name="atp", bufs=1) as atp:
            at_sb = atp.tile([FEAT, npc], F32, name="at_sb")
            nc.sync.dma_start(out=at_sb[:], in_=at_d[:])
            for j0 in range(0, npc, 512):
                jn = min(512, npc - j0)
                pe = ps_d.tile([128, 512], F32, tag="d", name="pe")
                nc.tensor.matmul(pe[:, :jn], embW_sb[:, :], at_sb[:, j0:j0 + jn],
                                 start=True, stop=False)
                nc.tensor.matmul(pe[:, :jn], embb_sb[:, :], mask_sb[:, j0:j0 + jn],
                                 start=False, stop=True)
                nc.scalar.copy(hT[:, j0:j0 + jn], pe[:, :jn])
        hfull = emit_h_rows()

        # ---- message passing layers ----
        for ll in range(L):
            l = ll % 3 if L > 3 else ll
            ci = 0  # call index
            for b in range(nb):
                blk = slice(b * 128, (b + 1) * 128)
                # S^T block accumulation over this block's gather calls
                block_calls = [c for c in calls if c[0] == b]
                n_block_chunks = sum(c[3] for c in block_calls) // 128
                psum_s = (ps_s.tile([128, 128], F32, tag="ps_s", name="psum_s")
                          if n_block_chunks else None)
                done = 0
                for (_, _, k, cc, off16, chunk0) in block_calls:
                    nch = cc // 128
                    g = gpool.tile([128, cfg.gcap // 128, 128], HDT, tag="g")
                    nc.gpsimd.dma_gather(
                        g[:, :nch, :],
                        hfull[k * cfg.bank_rows:(k + 1) * cfg.bank_rows, :],
                        idx_sb[:, off16: off16 + cc // 16],
                        num_idxs=cc, num_idxs_reg=cc, elem_size=HID,
                        queue_num=ci % cfg.n_queues,
                    )
                    ci += 1
                    for t in range(nch):
                        oh = ohpool.tile([128, 128], HDT, tag="oh")
                        nc.vector.tensor_scalar(
                            oh[:], iotah_sb[:], rel_sb[:, chunk0 + t: chunk0 + t + 1],
                            None, op0=mybir.AluOpType.is_equal)
                        nc.tensor.matmul(psum_s[:], g[:, t, :], oh[:],
                                         start=(done == 0),
                                         stop=(done == n_block_chunks - 1))
                        done += 1
                s_sb = spool.tile([128, 128], F32, tag="s")
                if n_block_chunks == 0:
                    nc.vector.memset(s_sb[:], 0.0)
                else:
                    nc.scalar.copy(s_sb[:], psum_s[:])

                # aggT = W_top^T S^T + W_bot^T (hT_blk * deg) + msg_b ⊗ deg
                hdeg_sb = spool.tile([128, 128], F32, tag="s", name="hdeg_sb")
                nc.vector.tensor_mul(hdeg_sb[:], hT[:, blk], degb_sb[:, blk])
                pa = ps_d.tile([128, 512], F32, tag="d")
                nc.tensor.matmul(pa[:, :128], mtop_sb[l][:], s_sb[:], start=True, stop=False)
                nc.tensor.matmul(pa[:, :128], mbot_sb[l][:], hdeg_sb[:], start=False, stop=False)
                nc.tensor.matmul(pa[:, :128], mb_sb[l][:], deg_sb[:, blk], start=False, stop=True)
                agg_sb = spool.tile([128, 128], F32, tag="s")
                nc.scalar.copy(agg_sb[:], pa[:, :128])

                # updT = U_top^T hT_blk + U_bot^T aggT + upd_b ⊗ mask
                pu = ps_d.tile([128, 512], F32, tag="d")
                nc.tensor.matmul(pu[:, :128], utop_sb[l][:], hT[:, blk], start=True, stop=False)
                nc.tensor.matmul(pu[:, :128], ubot_sb[l][:], agg_sb[:], start=False, stop=False)
                nc.tensor.matmul(pu[:, :128], ub_sb[l][:], mask_sb[:, blk], start=False, stop=True)
                relu_sb = spool.tile([128, 128], F32, tag="s")
                nc.scalar.activation(relu_sb[:], pu[:, :128],
                                     mybir.ActivationFunctionType.Relu)
                nc.vector.tensor_add(hT[:, blk], relu_sb[:], hT[:, blk])
            if ll < L - 1:
                hfull = emit_h_rows()

        # ---- readout: g = mean(h) ; out = MLP(g) ----
        part_sb = spool.tile([128, 1], F32, tag="s")
        nc.vector.tensor_reduce(part_sb[:], hT[:, :], axis=mybir.AxisListType.X,
                                op=mybir.AluOpType.add)
        part_dram = dram.tile([128, 1], F32, tag="pt_d")
        gsum_dram = dram.tile([128, 1], F32, tag="gs_d", addr_space="Shared")
        nc.sync.dma_start(out=part_dram[:], in_=part_sb[:])
        nc.gpsimd.collective_compute(
            "AllReduce", mybir.AluOpType.add,
            replica_groups=[list(range(C))],
            ins=[part_dram.opt()], outs=[gsum_dram.opt()],
        )
        gsum_sb = spool.tile([128, 1], F32, tag="s")
        nc.sync.dma_start(out=gsum_sb[:], in_=gsum_dram[:])

        p1 = ps_d.tile([128, 512], F32, tag="d")
        nc.tensor.matmul(p1[:HID // 2, :1], ow1_sb[:], gsum_sb[:], start=True, stop=True)
        o1_sb = spool.tile([HID // 2, 1], F32, tag="o1")
        nc.scalar.activation(o1_sb[:], p1[:HID // 2, :1],
                             mybir.ActivationFunctionType.Relu,
                             bias=ob1_sb[:], scale=1.0 / cfg.n_nodes)
        p2 = ps_d.tile([128, 512], F32, tag="d")
        nc.tensor.matmul(p2[:HID // 4, :1], ow2_sb[:], o1_sb[:], start=True, stop=True)
        o2_sb = spool.tile([HID // 4, 1], F32, tag="o2")
        nc.scalar.activation(o2_sb[:], p2[:HID // 4, :1],
                             mybir.ActivationFunctionType.Relu, bias=ob2_sb[:])
        p3 = ps_d.tile([128, 512], F32, tag="d")
        nc.tensor.matmul(p3[:1, :1], ow3_sb[:], o2_sb[:], start=True, stop=True)
        o3_sb = spool.tile([1, 1], F32, tag="o3")
        nc.scalar.activation(o3_sb[:], p3[:1, :1],
                             mybir.ActivationFunctionType.Identity, bias=ob3_sb[:])
        nc.sync.dma_start(out=out_d[:], in_=o3_sb[:])

    nc.compile()
    return nc


def make_in_maps(cfg: Cfg, inputs, per_core_arr):
    C, npc, FEAT, HID, L = cfg.n_cores, cfg.npc, cfg.feat, cfg.hid, cfg.layers
    af = np.asarray(inputs["atom_features"], np.float32)
    at_pad = np.zeros((cfg.n_pad, FEAT), np.float32)
    at_pad[:cfg.n_nodes] = af
    msg_W = np.asarray(inputs["msg_W"], np.float32)
    upd_W = np.asarray(inputs["upd_W"], np.float32)
    shared = dict(
        emb_W=np.asarray(inputs["emb_W"], np.float32),
        emb_b=np.asarray(inputs["emb_b"], np.float32)[None, :],
        msg_top=msg_W[:, :HID, :].reshape(L * HID, HID).copy(),
        msg_bot=msg_W[:, HID:, :].reshape(L * HID, HID).copy(),
        msg_bias=np.asarray(inputs["msg_b"], np.float32),
        upd_top=upd_W[:, :HID, :].reshape(L * HID, HID).copy(),
        upd_bot=upd_W[:, HID:, :].reshape(L * HID, HID).copy(),
        upd_bias=np.asarray(inputs["upd_b"], np.float32),
        out_W1=np.asarray(inputs["out_W1"], np.float32),
        out_b1=np.asarray(inputs["out_b1"], np.float32)[:, None],
        out_W2=np.asarray(inputs["out_W2"], np.float32),
        out_b2=np.asarray(inputs["out_b2"], np.float32)[:, None],
        out_W3=np.asarray(inputs["out_W3"], np.float32),
        out_b3=np.asarray(inputs["out_b3"], np.float32)[:, None],
    )
    in_maps = []
    for c in range(C):
        d = dict(shared)
        d["at_own"] = at_pad[c * npc:(c + 1) * npc].T.copy()
        d.update(per_core_arr[c])
        in_maps.append(d)
    return in_maps


_prog_cache = {}


def run(cfg: Cfg, inputs, trace=False):
    meta, per_core_arr = plan_edges(cfg, np.asarray(inputs["edge_indices"]))
    key = (cfg, hashlib.sha1(meta["caps"].tobytes()).hexdigest())
    if key not in _prog_cache:
        _prog_cache[key] = build_program(cfg, meta)
    nc = _prog_cache[key]
    in_maps = make_in_maps(cfg, inputs, per_core_arr)
    res = run_bass_kernel_spmd(nc, in_maps, core_ids=list(range(cfg.n_cores)),
                               trace=trace)
    out = res.results[0]["result"].astype(np.float32)
    return out, res


def kernel(**inputs) -> np.ndarray:
    out, _ = run(Cfg(), inputs)
    return out



# revision 20
# speedup vs baseline: 2.4236x; 2.4236x over previous
"""Trainium2 Bass kernel for EnhancedGNNModel (3-layer GNN message passing).

Strategy (8 NeuronCores, SPMD):
  - Nodes sharded by dst range: core c owns rows [c*NPC, (c+1)*NPC).
  - Edges sharded by dst; per-edge message Linear(concat(h_src, h_dst)) is
    split algebraically:
        msgs = h[src] @ W_top + h[dst] @ W_bot + b
    so  agg[d] = S[d] @ W_top + deg[d]*(h[d] @ W_bot) + deg[d]*b,
    with S[d] = sum of gathered h[src] rows (the only edge-granular work).
  - h rows (bf16) are AllGathered in two bank-halves per layer; dma_gather
    pulls h[src] rows for ~7 dst blocks at a time (one big SWDGE call per
    (block-group, bank) to amortize Q7 descriptor-generation overhead).
  - The scatter-add to S^T uses PE matmuls against one-hot(dst_rel) tiles
    built on the vector engine in 1x mode (F32 iota vs F32 rel -> BF16 out;
    mixed dtypes keep DVE out of the 2-port perf mode that would lock
    GpSimd's SWDGE descriptor writes), batched per gather call via
    stride-0 broadcast APs.
  - Dense per-node math runs in bf16, fused over 7-block groups (512/384
    wide matmuls); PSUM accumulates in f32.
  - AllGather half 0 is emitted mid-layer (after block 27) so the next
    layer's bank-0 gathers can start while half 1 is still in flight.
"""
import hashlib
import ml_dtypes
import numpy as np
from contextlib import ExitStack
from dataclasses import dataclass

import concourse.bass as bass
import concourse.tile as tile
from concourse import bacc, mybir
from concourse.bass_utils import run_bass_kernel_spmd

F32 = mybir.dt.float32
BF16 = mybir.dt.bfloat16
I16 = mybir.dt.int16


@dataclass(frozen=True)
class Cfg:
    n_nodes: int = 50000
    n_edges: int = 800000
    feat: int = 64
    hid: int = 128
    layers: int = 3
    n_cores: int = 8
    grp: int = 7       # dst blocks per gather group
    gcap: int = 1024   # max indices per dma_gather call (HW ucode cap)
    maxnch: int = 8    # gather tile capacity in 128-edge chunks (= gcap/128)
    dma_scratch: int = 65536
    n_queues: int = 4

    @property
    def npc(self):  # nodes per core (multiple of 128)
        per = -(-self.n_nodes // self.n_cores)
        return -(-per // 128) * 128

    @property
    def n_pad(self):
        return self.npc * self.n_cores

    @property
    def n_blocks(self):
        return self.npc // 128

    @property
    def n_groups(self):
        assert self.n_blocks % self.grp == 0
        return self.n_blocks // self.grp

    # bank 0 = local rows [0, half0_rows); bank 1 = rest. Half boundary is
    # block-aligned so emit/AllGather halves match gather banks exactly.
    @property
    def half0_blocks(self):
        return (self.n_blocks + 1) // 2

    @property
    def half0_rows(self):
        return self.half0_blocks * 128

    @property
    def half1_rows(self):
        return self.npc - self.half0_rows

    @property
    def bank_rows(self):  # gather-source rows per bank (all cores)
        b0 = self.half0_rows * self.n_cores
        b1 = self.half1_rows * self.n_cores
        assert b0 <= 32768 and b1 <= 32768, "dma_gather int16 index limit"
        return (b0, b1)


def _round_up(x, m):
    return -(-x // m) * m


def plan_edges(cfg: Cfg, edge_indices: np.ndarray):
    """Host-side plan. Returns (meta, per_core): meta is core-invariant
    program structure, per_core holds each core's data arrays."""
    src = np.asarray(edge_indices[0], dtype=np.int64)
    dst = np.asarray(edge_indices[1], dtype=np.int64)
    C, npc, nb, G = cfg.n_cores, cfg.npc, cfg.n_blocks, cfg.grp
    ngrp = cfg.n_groups
    h0r = cfg.half0_rows

    core = dst // npc
    ldst = dst - core * npc
    block = ldst >> 7
    rel = ldst & 127
    score = src // npc
    srow = src - score * npc
    bank = (srow >= h0r).astype(np.int64)
    bidx = np.where(bank == 1,
                    score * cfg.half1_rows + (srow - h0r),
                    score * h0r + srow)

    counts = np.zeros((C, nb, 2), dtype=np.int64)
    np.add.at(counts, (core, block, bank), 1)
    caps = counts.max(axis=0)
    caps = np.where(caps > 0, ((caps + 127) // 128) * 128, 0)  # [nb, 2]

    # order: (group, bank, block-within-group)
    group = block // G
    key = (group * 2 + bank) * G + (block - group * G)
    nkeys = ngrp * 2 * G

    def key_of(g, k, bi):
        return (g * 2 + k) * G + bi

    caps_key = np.zeros(nkeys, dtype=np.int64)
    for g in range(ngrp):
        for k in (0, 1):
            for bi in range(G):
                caps_key[key_of(g, k, bi)] = caps[g * G + bi, k]
    key_off = np.concatenate([[0], np.cumsum(caps_key)])[:-1]

    total_cap = int(caps.sum())
    n_chunks = total_cap // 128

    # chunk -> (group, local block, first-of-block, last-of-block)
    chunk_block = []          # local block index within group (0..G-1)
    chunk_flags = []          # (is_first_of_block, is_last_of_block)
    calls = []                # (bank, off16, chunk0, nch) per group: list

    calls_by_group = [[] for _ in range(ngrp)]
    off = 0
    chunk = 0
    for g in range(ngrp):
        # first/last chunk per block in this group (bank0 chunks then bank1)
        nch_b = [(int(caps[g * G + bi, 0]) // 128, int(caps[g * G + bi, 1]) // 128)
                 for bi in range(G)]
        seen = [0] * G
        tot_b = [a + b for a, b in nch_b]
        for k in (0, 1):
            # contiguous span of this (g, k): blocks in order
            span = sum(int(caps[g * G + bi, k]) for bi in range(G))
            done = 0
            while done < span:
                cc = min(cfg.gcap, span - done)
                calls_by_group[g].append((k, off // 16, chunk, cc // 128))
                # map this call's chunks to blocks
                off += cc
                chunk += cc // 128
                done += cc
            # chunk->block mapping for this (g,k) span
        # rebuild chunk_block/chunk_flags for the group's full chunk range
        for k in (0, 1):
            for bi in range(G):
                n = nch_b[bi][k]
                for _ in range(n):
                    chunk_block.append(bi)
                    first = seen[bi] == 0
                    seen[bi] += 1
                    last = seen[bi] == tot_b[bi]
                    chunk_flags.append((first, last))
    assert off == total_cap and chunk == n_chunks
    assert len(chunk_block) == n_chunks

    # per-core flat arrays in (g, k, b)-bucket order with cap padding
    per_core = []
    for c in range(C):
        m = core == c
        key_c, bidx_c, rel_c = key[m], bidx[m], rel[m]
        o = np.argsort(key_c, kind="stable")
        key_s, bidx_s, rel_s = key_c[o], bidx_c[o], rel_c[o]
        starts = np.concatenate([[0], np.cumsum(np.bincount(key_s, minlength=nkeys))])
        pos = key_off[key_s] + (np.arange(len(key_s)) - starts[key_s])
        idx_flat = np.zeros(total_cap, dtype=np.int16)
        rel_flat = np.full(total_cap, -1, dtype=np.int64)
        idx_flat[pos] = bidx_s.astype(np.int16)
        rel_flat[pos] = rel_s

        # wrap indices per call: idx j of a call sits at [j%16, j//16]
        idx_w = np.zeros((16, total_cap // 16), dtype=np.int16)
        for g in range(ngrp):
            for (k, off16, chunk0, nch) in calls_by_group[g]:
                cc = nch * 128
                seg = idx_flat[off16 * 16: off16 * 16 + cc]
                idx_w[:, off16: off16 + cc // 16] = seg.reshape(cc // 16, 16).T
        idx_w = np.tile(idx_w, (8, 1))

        # per-chunk dst_rel values [128, n_chunks]: col c, partition p = rel of
        # edge c*128+p (pad -1, one-hot built on DVE via is_equal vs iota)
        rel_t = rel_flat.reshape(n_chunks, 128).T.astype(np.float32).copy()

        deg = np.zeros(npc, dtype=np.float64)
        np.add.at(deg, ldst[m], 1.0)
        degb = np.tile(deg[None, :].astype(ml_dtypes.bfloat16), (128, 1))
        mask = np.zeros(npc, dtype=np.float64)
        lo = c * npc
        mask[: max(0, min(npc, cfg.n_nodes - lo))] = 1.0
        per_core.append(dict(idxs=idx_w, dstrel=rel_t,
                             mask=mask[None, :].astype(ml_dtypes.bfloat16),
                             deg_bcast=degb))

    meta = dict(caps=caps, calls_by_group=calls_by_group,
                chunk_block=chunk_block, chunk_flags=chunk_flags,
                n_chunks=n_chunks, total_cap=total_cap)
    return meta, per_core


def build_program(cfg: Cfg, meta):
    C, npc, nb, G, ngrp = cfg.n_cores, cfg.npc, cfg.n_blocks, cfg.grp, cfg.n_groups
    FEAT, HID, L = cfg.feat, cfg.hid, cfg.layers
    n_chunks, total_cap = meta["n_chunks"], meta["total_cap"]
    calls_by_group = meta["calls_by_group"]
    chunk_block, chunk_flags = meta["chunk_block"], meta["chunk_flags"]
    br0, br1 = cfg.bank_rows
    h0b = cfg.half0_blocks
    gw = G * 128  # group width in node columns (896)

    nc = bacc.Bacc("TRN2", target_bir_lowering=False, debug=False, num_devices=C,
                   dynamic_dma_scratch_size=cfg.dma_scratch,
                   num_swdge_queues=cfg.n_queues)

    def inp(name, shape, dt=BF16):
        return nc.dram_tensor(name, shape, dt, kind="ExternalInput").ap()

    at_d = inp("at_own", [FEAT, npc])
    embW_d = inp("emb_W", [FEAT, HID])
    embb_d = inp("emb_b", [1, HID])
    mtop_d = inp("msg_top", [L * HID, HID])
    mbot_d = inp("msg_bot", [L * HID, HID])
    mb_d = inp("msg_bias", [L, HID])
    utop_d = inp("upd_top", [L * HID, HID])
    ubot_d = inp("upd_bot", [L * HID, HID])
    ub_d = inp("upd_bias", [L, HID])
    ow1_d = inp("out_W1", [HID, HID // 2], F32)
    ob1_d = inp("out_b1", [HID // 2, 1], F32)
    ow2_d = inp("out_W2", [HID // 2, HID // 4], F32)
    ob2_d = inp("out_b2", [HID // 4, 1], F32)
    ow3_d = inp("out_W3", [HID // 4, 1], F32)
    ob3_d = inp("out_b3", [1, 1], F32)
    mask_d = inp("mask", [1, npc])
    degb_d = inp("deg_bcast", [128, npc])
    idx_d = inp("idxs", [128, total_cap // 16], I16)
    rel_d = inp("dstrel", [128, n_chunks], F32)
    out_d = nc.dram_tensor("result", [1, 1], F32, kind="ExternalOutput").ap()

    with tile.TileContext(nc) as tc, ExitStack() as ctx:
        const = ctx.enter_context(tc.tile_pool(name="const", bufs=1))
        gpool = ctx.enter_context(tc.tile_pool(name="g", bufs=6))
        ohpool = ctx.enter_context(tc.tile_pool(name="oh", bufs=6))
        spool = ctx.enter_context(tc.tile_pool(name="s", bufs=2))
        rpool = ctx.enter_context(tc.tile_pool(name="rows", bufs=3))
        ps_sc = ctx.enter_context(tc.tile_pool(name="ps_sc", bufs=2, space="PSUM"))
        pd = ctx.enter_context(tc.tile_pool(name="pd", bufs=3, space="PSUM"))
        pt_ps = ctx.enter_context(tc.tile_pool(name="pt_ps", bufs=1, space="PSUM"))
        dram = ctx.enter_context(tc.tile_pool(name="dram", bufs=2, space="DRAM"))

        def load_const(name, ap_dram, shape, dt=BF16):
            t = const.tile(shape, dt, name=name, tag=name)
            nc.sync.dma_start(out=t[:], in_=ap_dram)
            return t

        embW_sb = load_const("embW_sb", embW_d[:], [FEAT, HID])
        embb_sb = load_const("embb_sb", embb_d[:], [1, HID])
        mtop_sb = [load_const(f"mtop{l}", mtop_d[l * HID:(l + 1) * HID, :], [HID, HID]) for l in range(L)]
        mbot_sb = [load_const(f"mbot{l}", mbot_d[l * HID:(l + 1) * HID, :], [HID, HID]) for l in range(L)]
        mb_sb = [load_const(f"mb{l}", mb_d[l:l + 1, :], [1, HID]) for l in range(L)]
        utop_sb = [load_const(f"utop{l}", utop_d[l * HID:(l + 1) * HID, :], [HID, HID]) for l in range(L)]
        ubot_sb = [load_const(f"ubot{l}", ubot_d[l * HID:(l + 1) * HID, :], [HID, HID]) for l in range(L)]
        ub_sb = [load_const(f"ub{l}", ub_d[l:l + 1, :], [1, HID]) for l in range(L)]
        ow1_sb = load_const("ow1_sb", ow1_d[:], [HID, HID // 2], F32)
        ob1_sb = load_const("ob1_sb", ob1_d[:], [HID // 2, 1], F32)
        ow2_sb = load_const("ow2_sb", ow2_d[:], [HID // 2, HID // 4], F32)
        ob2_sb = load_const("ob2_sb", ob2_d[:], [HID // 4, 1], F32)
        ow3_sb = load_const("ow3_sb", ow3_d[:], [HID // 4, 1], F32)
        ob3_sb = load_const("ob3_sb", ob3_d[:], [1, 1], F32)
        mask_sb = load_const("mask_sb", mask_d[:], [1, npc])
        degb_sb = load_const("degb_sb", degb_d[:], [128, npc])
        deg_row = degb_sb[0:1, :]
        idx_sb = load_const("idx_sb", idx_d[:], [128, total_cap // 16], I16)
        rel_sb = load_const("rel_sb", rel_d[:], [128, n_chunks], F32)

        iota_sb = const.tile([128, 128], F32)
        nc.gpsimd.iota(iota_sb[:], [[1, 128]], channel_multiplier=0,
                       allow_small_or_imprecise_dtypes=True)
        iota_col = const.tile([128, 1], F32)
        nc.gpsimd.iota(iota_col[:], [[1, 1]], channel_multiplier=1,
                       allow_small_or_imprecise_dtypes=True)
        ident_sb = const.tile([128, 128], BF16)
        nc.vector.tensor_scalar(ident_sb[:], iota_sb[:], iota_col[:], None,
                                op0=mybir.AluOpType.is_equal)

        hT = const.tile([128, npc], BF16)    # h transposed, own nodes
        hdeg = const.tile([128, npc], BF16)  # h * deg (message bot term)

        rows_shape = (cfg.half0_rows, cfg.half1_rows)

        def emit_half(h):
            """Transpose updated hT rows of half h, DMA to DRAM, AllGather."""
            rows = rows_shape[h]
            b0 = 0 if h == 0 else h0b
            nblk = h0b if h == 0 else nb - h0b
            rows_dram = dram.tile([rows, HID], BF16, tag=f"rows_d{h}")
            for i in range(nblk):
                b = b0 + i
                blk = slice(b * 128, (b + 1) * 128)
                pt = pt_ps.tile([128, 512], BF16, tag="pt")
                nc.tensor.transpose(pt[:, :128], hT[:, blk], ident_sb[:])
                r_sb = rpool.tile([128, 128], BF16, tag="r")
                nc.scalar.copy(r_sb[:], pt[:, :128])
                nc.sync.dma_start(out=rows_dram[i * 128:(i + 1) * 128, :], in_=r_sb[:])
            hfull = dram.tile([rows * C, HID], BF16, tag=f"hfull_d{h}",
                              addr_space="Shared")
            nc.gpsimd.collective_compute(
                "AllGather", mybir.AluOpType.bypass,
                replica_groups=[list(range(C))],
                ins=[rows_dram.opt()], outs=[hfull.opt()],
            )
            return hfull

        # ---- embedding: hT = emb_W^T @ at_own + emb_b ⊗ mask ----
        with tc.tile_pool(name="atp", bufs=1) as atp:
            at_sb = atp.tile([FEAT, npc], BF16, name="at_sb")
            nc.sync.dma_start(out=at_sb[:], in_=at_d[:])
            for j0 in range(0, npc, 512):
                jn = min(512, npc - j0)
                pe = pd.tile([128, 512], F32, tag="pd")
                nc.tensor.matmul(pe[:, :jn], embW_sb[:, :], at_sb[:, j0:j0 + jn],
                                 start=True, stop=False)
                nc.tensor.matmul(pe[:, :jn], embb_sb[:, :], mask_sb[:, j0:j0 + jn],
                                 start=False, stop=True)
                nc.scalar.copy(hT[:, j0:j0 + jn], pe[:, :jn])
        cur = [emit_half(0), emit_half(1)]

        # ---- message passing layers ----
        # PSUM allows one accumulation chain per 2KB bank: compute bank-level
        # start/stop flags — the chronologically first/last chunk matmul
        # touching each of the group's two psum tiles (A: blocks 0-3, B: 4-6).
        # start lazily zeroes the whole bank; untouched sub-regions are
        # overwritten on first touch even with start=False.
        bank_flags = {}
        for g in range(ngrp):
            chunksA, chunksB = [], []
            for (k, off16, chunk0, nch) in calls_by_group[g]:
                for t in range(nch):
                    ci = chunk0 + t
                    (chunksA if chunk_block[ci] < 4 else chunksB).append(ci)
            d = {}
            for lst in (chunksA, chunksB):
                for ci in lst:
                    d[ci] = (False, False)
                if lst:
                    d[lst[0]] = (True, d[lst[0]][1])
                    d[lst[-1]] = (d[lst[-1]][0], True)
            bank_flags[g] = d

        for l in range(L):
            nxt = [None, None]
            nc.vector.tensor_mul(hdeg[:], hT[:], degb_sb[:])
            ps_tiles = {}
            qctr = [0]

            def do_calls(g, k, l=l):
                psA, psB = ps_tiles[g]
                for (kk, off16, chunk0, nch) in calls_by_group[g]:
                    if kk != k:
                        continue
                    cc = nch * 128
                    gt = gpool.tile([128, cfg.maxnch, 128], BF16, tag="g")
                    nc.gpsimd.dma_gather(
                        gt[:, :nch, :], cur[k][:],
                        idx_sb[:, off16: off16 + cc // 16],
                        num_idxs=cc, num_idxs_reg=cc, elem_size=HID,
                        queue_num=qctr[0] % cfg.n_queues,
                    )
                    qctr[0] += 1
                    # one-hot(dst_rel) built on DVE in 1x mode (F32 in, BF16
                    # out) — stride-0 broadcast APs batch all chunks in one op
                    oht = ohpool.tile([128, cfg.maxnch * 128], BF16, tag="oh")
                    io = iota_sb[:]
                    io3 = bass.AP(io.tensor, io.offset,
                                  [io.ap[0], [0, nch], io.ap[1]])
                    rl = rel_sb[:, chunk0: chunk0 + nch]
                    rl3 = bass.AP(rl.tensor, rl.offset,
                                  [rl.ap[0], rl.ap[1], [0, 128]])
                    oo = oht[:, :cc]
                    oo3 = bass.AP(oo.tensor, oo.offset,
                                  [oo.ap[0], [128, nch], [1, 128]])
                    nc.vector.tensor_tensor(oo3, io3, rl3,
                                            op=mybir.AluOpType.is_equal)
                    for t in range(nch):
                        bi = chunk_block[chunk0 + t]
                        first, last = bank_flags[g][chunk0 + t]
                        ps, poff = (psA, bi * 128) if bi < 4 else (psB, (bi - 4) * 128)
                        nc.tensor.matmul(ps[:, poff:poff + 128], gt[:, t, :],
                                         oht[:, t * 128:(t + 1) * 128],
                                         start=first, stop=last)

            def do_dense(g, l=l):
                psA, psB = ps_tiles.pop(g)
                s_g = spool.tile([128, gw], BF16, tag="s_g")
                nc.scalar.copy(s_g[:, :512], psA[:])
                nc.scalar.copy(s_g[:, 512:gw], psB[:, :gw - 512])
                for off, w in ((0, 512), (512, gw - 512)):
                    cols = slice(g * gw + off, g * gw + off + w)
                    pa = pd.tile([128, 512], F32, tag="pd")
                    nc.tensor.matmul(pa[:, :w], mtop_sb[l][:], s_g[:, off:off + w],
                                     start=True, stop=False)
                    nc.tensor.matmul(pa[:, :w], mbot_sb[l][:], hdeg[:, cols],
                                     start=False, stop=False)
                    nc.tensor.matmul(pa[:, :w], mb_sb[l][:], deg_row[:, cols],
                                     start=False, stop=True)
                    agg = spool.tile([128, 512], BF16, tag="agg")
                    nc.scalar.copy(agg[:, :w], pa[:, :w])
                    pu = pd.tile([128, 512], F32, tag="pd")
                    nc.tensor.matmul(pu[:, :w], utop_sb[l][:], hT[:, cols],
                                     start=True, stop=False)
                    nc.tensor.matmul(pu[:, :w], ubot_sb[l][:], agg[:, :w],
                                     start=False, stop=False)
                    nc.tensor.matmul(pu[:, :w], ub_sb[l][:], mask_sb[:, cols],
                                     start=False, stop=True)
                    relu = spool.tile([128, 512], BF16, tag="relu")
                    nc.scalar.activation(relu[:, :w], pu[:, :w],
                                         mybir.ActivationFunctionType.Relu)
                    nc.vector.tensor_add(hT[:, cols], relu[:, :w], hT[:, cols])

            # bank-0 calls run one group ahead of bank-1 calls so GpSimd never
            # head-of-line blocks on the half-1 AllGather at layer entry.
            for g in range(ngrp + 1):
                if g < ngrp:
                    ps_tiles[g] = (ps_sc.tile([128, 512], F32, tag="psA", name="psA"),
                                   ps_sc.tile([128, 512], F32, tag="psB", name="psB"))
                    do_calls(g, 0)
                if g >= 1:
                    do_calls(g - 1, 1)
                    do_dense(g - 1)
                    if l < L - 1 and g - 1 == 3:
                        nxt[0] = emit_half(0)
            if l < L - 1:
                nxt[1] = emit_half(1)
                cur = nxt

        # ---- readout: g = mean(h) ; out = MLP(g) ----
        part_sb = spool.tile([128, 1], F32, tag="part")
        nc.vector.tensor_reduce(part_sb[:], hT[:, :], axis=mybir.AxisListType.X,
                                op=mybir.AluOpType.add)
        part_dram = dram.tile([128, 1], F32, tag="pt_d")
        gsum_dram = dram.tile([128, 1], F32, tag="gs_d", addr_space="Shared")
        nc.sync.dma_start(out=part_dram[:], in_=part_sb[:])
        nc.gpsimd.collective_compute(
            "AllReduce", mybir.AluOpType.add,
            replica_groups=[list(range(C))],
            ins=[part_dram.opt()], outs=[gsum_dram.opt()],
        )
        gsum_sb = spool.tile([128, 1], F32, tag="gsum")
        nc.sync.dma_start(out=gsum_sb[:], in_=gsum_dram[:])

        p1 = pd.tile([128, 512], F32, tag="pd")
        nc.tensor.matmul(p1[:HID // 2, :1], ow1_sb[:], gsum_sb[:], start=True, stop=True)
        o1_sb = spool.tile([HID // 2, 1], F32, tag="o1")
        nc.scalar.activation(o1_sb[:], p1[:HID // 2, :1],
                             mybir.ActivationFunctionType.Relu,
                             bias=ob1_sb[:], scale=1.0 / cfg.n_nodes)
        p2 = pd.tile([128, 512], F32, tag="pd")
        nc.tensor.matmul(p2[:HID // 4, :1], ow2_sb[:], o1_sb[:], start=True, stop=True)
        o2_sb = spool.tile([HID // 4, 1], F32, tag="o2")
        nc.scalar.activation(o2_sb[:], p2[:HID // 4, :1],
                             mybir.ActivationFunctionType.Relu, bias=ob2_sb[:])
        p3 = pd.tile([128, 512], F32, tag="pd")
        nc.tensor.matmul(p3[:1, :1], ow3_sb[:], o2_sb[:], start=True, stop=True)
        o3_sb = spool.tile([1, 1], F32, tag="o3")
        nc.scalar.activation(o3_sb[:], p3[:1, :1],
                             mybir.ActivationFunctionType.Identity, bias=ob3_sb[:])
        nc.sync.dma_start(out=out_d[:], in_=o3_sb[:])

    nc.compile()
    return nc


def _bf(x):
    return np.asarray(x, np.float32).astype(ml_dtypes.bfloat16)


def make_in_maps(cfg: Cfg, inputs, per_core_arr):
    C, npc, FEAT, HID, L = cfg.n_cores, cfg.npc, cfg.feat, cfg.hid, cfg.layers
    af = np.asarray(inputs["atom_features"], np.float32)
    at_pad = np.zeros((cfg.n_pad, FEAT), np.float32)
    at_pad[:cfg.n_nodes] = af
    msg_W = np.asarray(inputs["msg_W"], np.float32)
    upd_W = np.asarray(inputs["upd_W"], np.float32)
    shared = dict(
        emb_W=_bf(inputs["emb_W"]),
        emb_b=_bf(inputs["emb_b"])[None, :],
        msg_top=_bf(msg_W[:, :HID, :].reshape(L * HID, HID)),
        msg_bot=_bf(msg_W[:, HID:, :].reshape(L * HID, HID)),
        msg_bias=_bf(inputs["msg_b"]),
        upd_top=_bf(upd_W[:, :HID, :].reshape(L * HID, HID)),
        upd_bot=_bf(upd_W[:, HID:, :].reshape(L * HID, HID)),
        upd_bias=_bf(inputs["upd_b"]),
        out_W1=np.asarray(inputs["out_W1"], np.float32),
        out_b1=np.asarray(inputs["out_b1"], np.float32)[:, None],
        out_W2=np.asarray(inputs["out_W2"], np.float32),
        out_b2=np.asarray(inputs["out_b2"], np.float32)[:, None],
        out_W3=np.asarray(inputs["out_W3"], np.float32),
        out_b3=np.asarray(inputs["out_b3"], np.float32)[:, None],
    )
    in_maps = []
    for c in range(C):
        d = dict(shared)
        d["at_own"] = _bf(at_pad[c * npc:(c + 1) * npc].T.copy())
        d.update(per_core_arr[c])
        in_maps.append(d)
    return in_maps


_prog_cache = {}


def run(cfg: Cfg, inputs, trace=False):
    meta, per_core_arr = plan_edges(cfg, np.asarray(inputs["edge_indices"]))
    key = (cfg, hashlib.sha1(meta["caps"].tobytes()).hexdigest())
    if key not in _prog_cache:
        _prog_cache[key] = build_program(cfg, meta)
    nc = _prog_cache[key]
    in_maps = make_in_maps(cfg, inputs, per_core_arr)
    res = run_bass_kernel_spmd(nc, in_maps, core_ids=list(range(cfg.n_cores)),
                               trace=trace)
    out = res.results[0]["result"].astype(np.float32)
    return out, res


def kernel(**inputs) -> np.ndarray:
    out, _ = run(Cfg(), inputs)
    return out
